# revision 15
# baseline (speedup 1.0000x reference)
"""CapsuleLinear dynamic-routing kernel for TRN2 (8 NeuronCores, data-parallel over batch).

Math (reference):
    priors[n,j,i,k] = sum_l x[n,i,l] * w[j,k,l]          (never materialized)
    3 routing iterations entirely in the L=8 compressed space:
      probs = softmax_j(logits)                          logits[n,i,j], init 0
      s[n,j,l]  = sum_i probs[n,j,i] * x[n,i,l]          (PE matmul, contraction over i)
      v'[n,j,l] = sum_l' G[j,l,l'] s[n,j,l']             (G = W^T W, host-precomputed)
      sq[n,j]   = s . v'  (= ||W s||^2)
      v[n,j,l]  = scale(sq) * v'                         (squash scale folded in)
      logits   += sum_l x[n,i,l] * v[n,j,l]              (PE matmul, PSUM-resident accum)
    Final iter only: u = W s, out = scale(sq) * u.

Layout: i = 9*p + q  (p = SBUF partition 0..127, q = 0..8).
Per-sample tensors are packed (n2, j) on 128 partitions (n = 2h + n2, h = sample
half).  The (a) matmuls are emitted one-per-sample (4 per q) so s lands
full-partition-width at column 0 of its PSUM bank — the squash chain reads it
straight from PSUM with no diagonal-extract copies.
logits lives in 5 PSUM tiles of one bank each (q pairs); softmax d/r/xs are
processed per q-pair group so the exp of early groups feeds (a) matmuls while
later groups are still in flight.  Softmax normalization is folded into xpad
(x * r per i); x/J in bf16 for iter 0 (J folded back via the Exp bias ln(J)).
Matmul operands are bf16; all PSUM accumulation is f32.
"""

import os

import numpy as np

N, I, L, J, K = 32, 1152, 8, 64, 16
NCORES = 8
NPC = N // NCORES  # samples per core = 4
P = 128
Q = I // P  # 9
ITERS = 3
EPS = 1e-9
LN_J = float(np.log(float(J)))
NG = 5  # exp/logit PSUM groups: q pairs (0,1),(2,3),(4,5),(6,7),(8,)

_cache = {}
LAST_RESULT = None


def _patch_act_tables():
    """Restrict every activation to the one table set containing Exp+Ln
    (+Copy/Identity/Square), so bacc emits a single ACT_TABLE_LOAD."""
    import concourse.hw_specs as hw_specs

    import concourse.bacc as bacc

    if getattr(hw_specs, "_capsule_patched", False):
        return
    orig = hw_specs.get_activation_tables

    def patched(arch):
        t = dict(orig(arch))
        both = "natural_log_exp_and_others"
        if both in t:
            keep = t[both]
            for name in t:
                if name != both:
                    t[name] = t[name] - keep
        return t

    hw_specs.get_activation_tables = patched
    bacc.get_activation_tables = patched  # bacc binds the name via from-import
    hw_specs._capsule_patched = True


def _build():
    import concourse.bacc as bacc
    import concourse.tile as tile
    from concourse import mybir
    from concourse.masks import make_identity

    _patch_act_tables()

    f32 = mybir.dt.float32
    bf16 = mybir.dt.bfloat16
    AF = mybir.ActivationFunctionType
    AX = mybir.AxisListType
    OP = mybir.AluOpType
    dlow = bool(int(os.environ.get("DLOW", "0")))
    ddt = bf16 if dlow else f32

    nc = bacc.Bacc("TRN2", target_bir_lowering=False, debug=False, num_devices=NCORES)

    x_d = nc.dram_tensor("x", (NPC, I, L), f32, kind="ExternalInput")
    w_d = nc.dram_tensor("weight", (P, K, L), f32, kind="ExternalInput")  # replicated
    g_d = nc.dram_tensor("gram", (P, L, L), f32, kind="ExternalInput")  # replicated
    o_d = nc.dram_tensor("out", (NPC, J, K), f32, kind="ExternalOutput")

    with tile.TileContext(nc) as tc:
        with tc.tile_pool(name="singles", bufs=1) as singles:
            # ---- input DMAs: 2-way sample split; w/G single-trigger ----
            xall = singles.tile([P, NPC, Q, L], f32)
            xsrc = x_d[:].rearrange("n (p q) l -> p n q l", p=P)
            nc.sync.dma_start(out=xall[:, 0:2], in_=xsrc[:, 0:2])
            nc.scalar.dma_start(out=xall[:, 2:4], in_=xsrc[:, 2:4])
            w2 = singles.tile([P, K, L], f32)
            nc.sync.dma_start(out=w2[:], in_=w_d[:])
            g2 = singles.tile([P, L, L], f32)
            nc.scalar.dma_start(out=g2[:], in_=g_d[:])

            # ---- constants (xpad memset first: it gates the xs converts) ----
            xpad = singles.tile([P, Q, NPC, 32], bf16)
            nc.gpsimd.memset(xpad, 0.0)
            id_t = singles.tile([P, P], bf16)
            make_identity(nc, id_t)
            vT_pad = singles.tile([P, 2, 32], bf16)
            nc.gpsimd.memset(vT_pad, 0.0)
            # block-diag v: sample n occupies partitions 32n..32n+8
            vblk = singles.tile([P, NPC * J], bf16)
            nc.gpsimd.memset(vblk, 0.0)
            eps_t = singles.tile([P, 1], f32)
            nc.gpsimd.memset(eps_t, EPS)
            lnj_t = singles.tile([P, 1], f32)
            nc.gpsimd.memset(lnj_t, LN_J)

            # warm the ACT ln/exp table set while DMAs run
            warm = singles.tile([1, 2], f32)
            nc.gpsimd.memset(warm, 1.0)
            nc.scalar.activation(warm[:, 0:1], warm[:, 0:1], AF.Ln)
            nc.scalar.activation(warm[:, 1:2], warm[:, 1:2], AF.Exp)

            # ---- per-sample bf16 conversion (iter-0 probs folded: x/J) ----
            for n in range(NPC):
                nc.vector.tensor_scalar_mul(
                    xpad[:, :, n : n + 1, 0:L],
                    xall[:, n : n + 1].transpose([0, 2, 1, 3]),
                    1.0 / J,
                )

            # ---- iteration temporaries ----
            xT_sb = singles.tile([P, Q, P], bf16)
            e_t = singles.tile([P, Q, NPC, J], bf16)
            d_t = singles.tile([P, Q, NPC], ddt)
            r_t = singles.tile([P, Q, NPC], ddt)
            gt2 = singles.tile([P, 2, L, L], f32)
            vpr = singles.tile([P, 2, L], f32)
            svp = singles.tile([P, 2, L], f32)
            sq2 = singles.tile([P, 2], f32)
            ln2 = singles.tile([P, 2], f32)
            rti = singles.tile([P, 2], f32)
            sp1 = singles.tile([P, 2], f32)
            r1 = singles.tile([P, 2], f32)
            m2a = singles.tile([P, 2], f32)
            m2 = singles.tile([P, 2], f32)
            pt2 = singles.tile([P, 2, K, L], f32)
            u2 = singles.tile([P, 2, K], f32)
            oc = singles.tile([P, 2, K], f32)

            def groups():
                for g in range(NG):
                    yield g, (2 if g < 4 else 1)

            def a_phase(t):
                """(a) matmuls, one per sample: s for half h lands at
                s2h[h][:, 0:L] across the full 128 partitions (n2 blocks),
                so the squash chain reads PSUM directly (no diag copies)."""
                s2h = [
                    s2a_pool.tile([P, L], f32, tag="s2a", name="s2a"),
                    s2b_pool.tile([P, L], f32, tag="s2b", name="s2b"),
                ]
                for h in range(2):
                    for q in range(Q):
                        for n2 in range(2):
                            n = 2 * h + n2
                            lhsT = (
                                ones_h[:, 0:64]
                                if t == 0
                                else e_t[:, q, n, :]
                            )
                            nc.tensor.matmul(
                                s2h[h][64 * n2 : 64 * n2 + 64, :],
                                lhsT,
                                xpad[:, q, n, 0:L],
                                start=(q == 0),
                                stop=(q == Q - 1),
                                skip_group_check=True,
                            )
                return s2h

            def v_phase(t, s2h):
                """v' = G s, sq = s.v', scale chain into vT_pad (scale*J).
                Vector-centric; scalar only does Ln/Exp."""
                for h in range(2):
                    nc.vector.tensor_mul(
                        gt2[:, h],
                        g2[:],
                        s2h[h][:].unsqueeze(1).broadcast_to((P, L, L)),
                    )
                nc.vector.reduce_sum(vpr, gt2, axis=AX.X)
                for h in range(2):
                    nc.vector.tensor_mul(svp[:, h], s2h[h][:], vpr[:, h])
                nc.vector.tensor_reduce(sq2, svp, axis=AX.X, op=OP.add)
                # scale = sq/((1+sq) sqrt(sq+eps)); sqrt via exp(-0.5 ln);
                # ln(J) bias folds away the 1/J baked into xT
                nc.scalar.activation(ln2, sq2, AF.Ln, bias=eps_t[:])
                nc.scalar.activation(rti, ln2, AF.Exp, scale=-0.5, bias=lnj_t[:])
                nc.vector.tensor_scalar_add(sp1, sq2, 1.0)
                nc.vector.reciprocal(r1, sp1)
                nc.vector.tensor_mul(m2a, sq2, r1)
                nc.vector.tensor_mul(m2, m2a, rti)
                nc.vector.tensor_mul(
                    vT_pad[:, :, 0:L],
                    vpr,
                    m2[:].unsqueeze(2).broadcast_to((P, 2, L)),
                )

            def vtr_phase(t):
                """PE transpose of vT_pad; scatter sample n to vblk rows 32n."""
                vtr = vtr_pool.tile([2 * 32, P], bf16, tag="vtr", name="vtr")
                nc.tensor.transpose(
                    vtr, vT_pad[:].rearrange("p h w -> p (h w)"), id_t
                )
                for n in range(NPC):
                    h, n2 = n // 2, n % 2
                    src = vtr[32 * h : 32 * h + L, 64 * n2 : 64 * n2 + 64]
                    dst = vblk[32 * n : 32 * n + L, 64 * n : 64 * n + 64]
                    if n % 2 == 0:
                        nc.vector.tensor_copy(dst, src)
                    else:
                        nc.scalar.copy(dst, src)

            ones_h = singles.tile([P, 64], bf16)
            nc.gpsimd.memset(ones_h, 1.0)

            with tc.tile_pool(name="s2a_ps", bufs=1, space="PSUM") as s2a_pool, \
                 tc.tile_pool(name="s2b_ps", bufs=1, space="PSUM") as s2b_pool, \
                 tc.tile_pool(name="vtr_ps", bufs=1, space="PSUM") as vtr_pool:

                # ================= iteration 0 =================
                with tc.tile_pool(name="xtp_ps", bufs=2, space="PSUM") as xtp_pool:
                    s2h0 = a_phase(0)
                    # xT[32n+l, q, p] = x[n, 9p+q, l]/J via 9 PE transposes;
                    # PSUM->SBUF copies alternate vector/scalar
                    for q in range(Q):
                        xtp = xtp_pool.tile([P, P], bf16, tag="xtp", name="xtp")
                        nc.tensor.transpose(
                            xtp, xpad[:, q].rearrange("p n w -> p (n w)"), id_t
                        )
                        if q % 2 == 0:
                            nc.vector.tensor_copy(xT_sb[:, q, :], xtp)
                        else:
                            nc.scalar.copy(xT_sb[:, q, :], xtp)
                    v_phase(0, s2h0)
                    vtr_phase(0)

                # ================= iterations 1..2 =================
                # logits as 5 one-bank PSUM tiles (q pairs) so the exp of early
                # pairs only waits on their own (b) matmuls
                with tc.tile_pool(name="lp_ps", bufs=1, space="PSUM") as lp_pool:
                    lp = []
                    for g, nq in groups():
                        lpt = lp_pool.tile(
                            [P, nq, NPC, J], f32, tag=f"lp{g}", name=f"lp{g}"
                        )
                        lp.append(lpt)

                    for t in range(1, ITERS):
                        # ---- (b) matmuls with prev iter's vblk ----
                        for q in range(Q):
                            g, qq = q // 2, q % 2
                            nc.tensor.matmul(
                                lp[g][:, qq].rearrange("p n j -> p (n j)"),
                                xT_sb[:, q, :],
                                vblk[:],
                                start=(t == 1 and qq == 0),
                                stop=(t == ITERS - 1 and (qq == 1 or q == Q - 1)),
                                skip_group_check=True,
                            )
                        # ---- softmax per q-pair group: exp, d, r, xpad=x*r ----
                        for g, nq in groups():
                            sl = slice(2 * g, 2 * g + nq)
                            nc.scalar.activation(e_t[:, sl], lp[g][:], AF.Exp)
                            nc.vector.tensor_reduce(
                                d_t[:, sl], e_t[:, sl], axis=AX.X, op=OP.add
                            )
                            nc.vector.reciprocal(r_t[:, sl], d_t[:, sl])
                            nc.gpsimd.tensor_mul(
                                xpad[:, sl, :, 0:L],
                                xall[:, :, sl, :].transpose([0, 2, 1, 3]),
                                r_t[:, sl]
                                .unsqueeze(3)
                                .broadcast_to((P, nq, NPC, L)),
                            )

                        s2h = a_phase(t)

                        if t < ITERS - 1:
                            v_phase(t, s2h)
                            vtr_phase(t)
                        else:
                            # ---- final: sq via G (feeds scalar), u = W s ----
                            for h in range(2):
                                nc.vector.tensor_mul(
                                    gt2[:, h],
                                    g2[:],
                                    s2h[h][:].unsqueeze(1).broadcast_to((P, L, L)),
                                )
                            nc.vector.reduce_sum(vpr, gt2, axis=AX.X)
                            for h in range(2):
                                nc.vector.tensor_mul(
                                    svp[:, h], s2h[h][:], vpr[:, h]
                                )
                            nc.vector.tensor_reduce(sq2, svp, axis=AX.X, op=OP.add)
                            nc.scalar.activation(ln2, sq2, AF.Ln, bias=eps_t[:])
                            nc.scalar.activation(rti, ln2, AF.Exp, scale=-0.5)
                            for h in range(2):
                                nc.vector.tensor_mul(
                                    pt2[:, h],
                                    w2[:],
                                    s2h[h][:].unsqueeze(1).broadcast_to((P, K, L)),
                                )
                            nc.vector.reduce_sum(u2, pt2, axis=AX.X)
                            nc.vector.tensor_scalar_add(sp1, sq2, 1.0)
                            nc.vector.reciprocal(r1, sp1)
                            nc.vector.tensor_mul(m2a, sq2, r1)
                            nc.vector.tensor_mul(m2, m2a, rti)
                            nc.vector.tensor_mul(
                                oc, u2, m2[:].unsqueeze(2).broadcast_to((P, 2, K))
                            )
                            # oc[(n2 j), h, k] -> out[n, j, k], n = 2h + n2
                            nc.sync.dma_start(
                                out=o_d[:].rearrange(
                                    "(h n2) j k -> (n2 j) h k", h=2
                                ),
                                in_=oc,
                            )

    nc.finalize()
    return nc


def _warm_axon():
    """Run one trivial op on the axon devices so the PJRT client (and the
    NTFF profile sidechannel) is fully initialized before the traced run."""
    if _cache.get("warm"):
        return
    try:
        import jax
        import jax.numpy as jnp

        d = jax.devices()[0]
        jnp.add(jax.device_put(jnp.ones((1,), jnp.float32), d), 1.0).block_until_ready()
    except Exception:
        pass
    _cache["warm"] = True


def kernel(x, weight):
    global LAST_RESULT
    from concourse.bass_utils import run_bass_kernel_spmd

    if "nc" not in _cache:
        _cache["nc"] = _build()
    nc = _cache["nc"]
    _warm_axon()

    x = np.ascontiguousarray(np.asarray(x, dtype=np.float32))
    weight = np.ascontiguousarray(np.asarray(weight, dtype=np.float32))
    gram = np.einsum("jkl,jkm->jlm", weight, weight)
    w_rep = np.ascontiguousarray(np.concatenate([weight, weight], axis=0))
    g_rep = np.ascontiguousarray(
        np.concatenate([gram, gram], axis=0).astype(np.float32)
    )

    in_maps = [
        {"x": x[c * NPC : (c + 1) * NPC], "weight": w_rep, "gram": g_rep}
        for c in range(NCORES)
    ]
    last_exc = None
    for attempt in range(3):
        try:
            res = run_bass_kernel_spmd(nc, in_maps, core_ids=list(range(NCORES)))
            break
        except Exception as e:
            last_exc = e
            import time

            time.sleep(5 * (attempt + 1))
    else:
        raise last_exc
    LAST_RESULT = res
    return np.concatenate([r["out"] for r in res.results], axis=0)


# revision 22
# speedup vs baseline: 1.0369x; 1.0369x over previous
"""CapsuleLinear dynamic-routing kernel for TRN2 (8 NeuronCores, data-parallel over batch).

Math (reference):
    priors[n,j,i,k] = sum_l x[n,i,l] * w[j,k,l]          (never materialized)
    3 routing iterations entirely in the L=8 compressed space:
      probs = softmax_j(logits)                          logits[n,i,j], init 0
      s[n,j,l]  = sum_i probs[n,j,i] * x[n,i,l]          (PE matmul, contraction over i)
      v'[n,j,l] = sum_l' G[j,l,l'] s[n,j,l']             (G = W^T W, host-precomputed)
      sq[n,j]   = s . v'  (= ||W s||^2)
      v[n,j,l]  = scale(sq) * v'                         (squash scale folded in)
      logits   += sum_l x[n,i,l] * v[n,j,l]              (PE matmul, PSUM-resident accum)
    Final iter only: u = W s, out = scale(sq) * u.

Layout: i = 9*p + q  (p = SBUF partition 0..127, q = 0..8).
Per-sample tensors are packed (n2, j) on 128 partitions (n = 2h + n2, h = sample
half).  The (a) matmuls are emitted one-per-sample (4 per q) so s lands
full-partition-width at column 0 of its PSUM bank — the squash chain reads it
straight from PSUM with no diagonal-extract copies.
logits lives in 5 PSUM tiles of one bank each (q pairs); softmax d/r/xs are
processed per q-pair group so the exp of early groups feeds (a) matmuls while
later groups are still in flight.  Softmax normalization is folded into xpad
(x * r per i); x/J in bf16 for iter 0 (J folded back via the Exp bias ln(J)).
Matmul operands are bf16; all PSUM accumulation is f32.
"""

import os

import numpy as np

N, I, L, J, K = 32, 1152, 8, 64, 16
NCORES = 8
NPC = N // NCORES  # samples per core = 4
P = 128
Q = I // P  # 9
ITERS = 3
EPS = 1e-9
LN_J = float(np.log(float(J)))
NG = 5  # exp/logit PSUM groups: q pairs (0,1),(2,3),(4,5),(6,7),(8,)

_cache = {}
LAST_RESULT = None


def _patch_act_tables():
    """Restrict every activation to the one table set containing Exp+Ln
    (+Copy/Identity/Square), so bacc emits a single ACT_TABLE_LOAD."""
    import concourse.hw_specs as hw_specs

    import concourse.bacc as bacc

    if getattr(hw_specs, "_capsule_patched", False):
        return
    orig = hw_specs.get_activation_tables

    def patched(arch):
        t = dict(orig(arch))
        both = "natural_log_exp_and_others"
        if both in t:
            keep = t[both]
            for name in t:
                if name != both:
                    t[name] = t[name] - keep
        return t

    hw_specs.get_activation_tables = patched
    bacc.get_activation_tables = patched  # bacc binds the name via from-import
    hw_specs._capsule_patched = True


def _build():
    import concourse.bacc as bacc
    import concourse.tile as tile
    from concourse import mybir
    from concourse.masks import make_identity

    _patch_act_tables()

    f32 = mybir.dt.float32
    bf16 = mybir.dt.bfloat16
    AF = mybir.ActivationFunctionType
    AX = mybir.AxisListType
    OP = mybir.AluOpType
    dlow = bool(int(os.environ.get("DLOW", "0")))
    ddt = bf16 if dlow else f32

    nc = bacc.Bacc("TRN2", target_bir_lowering=False, debug=False, num_devices=NCORES)

    x_d = nc.dram_tensor("x", (NPC, I, L), f32, kind="ExternalInput")
    # weight/gram replicated to 128 partitions host-side, shipped bf16
    w_d = nc.dram_tensor("weight", (P, K, L), bf16, kind="ExternalInput")
    g_d = nc.dram_tensor("gram", (P, L, L), bf16, kind="ExternalInput")
    o_d = nc.dram_tensor("out", (NPC, J, K), f32, kind="ExternalOutput")

    with tile.TileContext(nc) as tc:
        with tc.tile_pool(name="singles", bufs=1) as singles:
            # ---- input DMAs: 2-way sample split; w/G single-trigger ----
            xall = singles.tile([P, NPC, Q, L], f32)
            xsrc = x_d[:].rearrange("n (p q) l -> p n q l", p=P)
            nc.sync.dma_start(out=xall[:, 0:2], in_=xsrc[:, 0:2])
            nc.scalar.dma_start(out=xall[:, 2:4], in_=xsrc[:, 2:4])
            w2 = singles.tile([P, K, L], bf16)
            nc.sync.dma_start(out=w2[:], in_=w_d[:])
            g2 = singles.tile([P, L, L], bf16)
            nc.scalar.dma_start(out=g2[:], in_=g_d[:])

            # ---- constants (xpad memset first: it gates the xs converts) ----
            xpad = singles.tile([P, Q, NPC, 32], bf16)
            nc.gpsimd.memset(xpad, 0.0)
            id_t = singles.tile([P, P], bf16)
            make_identity(nc, id_t)
            vT_pad = singles.tile([P, 2, 32], bf16)
            nc.gpsimd.memset(vT_pad, 0.0)
            # block-diag v: sample n occupies partitions 32n..32n+8
            vblk = singles.tile([P, NPC * J], bf16)
            nc.gpsimd.memset(vblk, 0.0)
            eps_t = singles.tile([P, 1], f32)
            nc.gpsimd.memset(eps_t, EPS)
            lnj_t = singles.tile([P, 1], f32)
            nc.gpsimd.memset(lnj_t, LN_J)

            # warm the ACT ln/exp table set while DMAs run
            warm = singles.tile([1, 2], f32)
            nc.gpsimd.memset(warm, 1.0)
            nc.scalar.activation(warm[:, 0:1], warm[:, 0:1], AF.Ln)
            nc.scalar.activation(warm[:, 1:2], warm[:, 1:2], AF.Exp)

            # ---- per-sample bf16 conversion (iter-0 probs folded: x/J) ----
            for n in range(NPC):
                nc.vector.tensor_scalar_mul(
                    xpad[:, :, n : n + 1, 0:L],
                    xall[:, n : n + 1].transpose([0, 2, 1, 3]),
                    1.0 / J,
                )

            # ---- iteration temporaries ----
            xT_sb = singles.tile([P, Q, P], bf16)
            e_t = singles.tile([P, Q, NPC, J], bf16)
            d_t = singles.tile([P, Q, NPC], ddt)
            r_t = singles.tile([P, Q, NPC], ddt)
            gt2 = singles.tile([P, 2, L, L], f32)
            vpr = singles.tile([P, 2, L], f32)
            svp = singles.tile([P, 2, L], f32)
            sq2 = singles.tile([P, 2], f32)
            ln2 = singles.tile([P, 2], f32)
            rti = singles.tile([P, 2], f32)
            sp1 = singles.tile([P, 2], f32)
            r1 = singles.tile([P, 2], f32)
            m2a = singles.tile([P, 2], f32)
            m2 = singles.tile([P, 2], f32)
            pt2 = singles.tile([P, 2, K, L], f32)
            u2 = singles.tile([P, 2, K], f32)
            oc = singles.tile([P, 2, K], f32)

            def groups():
                for g in range(NG):
                    yield g, (2 if g < 4 else 1)

            def a_phase(t):
                """(a) matmuls into s2a/s2b PSUM banks.
                t=0: probs uniform -> 2 wide ones-matmuls per q (s packed
                (n2, l) in 16 cols, diag extracted by the v-chain APs).
                t>=1: one matmul per sample (4 per q) so s lands at cols 0:L
                across the full partition width (no diag extract at all)."""
                s2h = [
                    s2a_pool.tile([P, 2 * L], f32, tag="s2a", name="s2a"),
                    s2b_pool.tile([P, 2 * L], f32, tag="s2b", name="s2b"),
                ]
                for h in range(2):
                    for q in range(Q):
                        if t == 0:
                            nc.tensor.matmul(
                                s2h[h],
                                ones_t[:],
                                xpad[:, q, 2 * h : 2 * h + 2, 0:L],
                                start=(q == 0),
                                stop=(q == Q - 1),
                                skip_group_check=True,
                            )
                        else:
                            for n2 in range(2):
                                n = 2 * h + n2
                                nc.tensor.matmul(
                                    s2h[h][64 * n2 : 64 * n2 + 64, 0:L],
                                    e_t[:, q, n, :],
                                    xpad[:, q, n, 0:L],
                                    start=(q == 0),
                                    stop=(q == Q - 1),
                                    skip_group_check=True,
                                )
                return s2h

            def s_aps(t, s2h, h):
                """Where s for half h lives in PSUM: (row-slice, AP) pieces."""
                if t == 0:
                    return [
                        (slice(0, 64), s2h[h][0:64, 0:L]),
                        (slice(64, 128), s2h[h][64:128, L : 2 * L]),
                    ]
                return [(slice(0, P), s2h[h][:, 0:L])]

            def v_phase(t, s2h):
                """v' = G s, sq = s.v', scale chain into vT_pad (scale*J).
                Vector-centric; scalar only does Ln/Exp."""
                for h in range(2):
                    for rows, ap in s_aps(t, s2h, h):
                        nr = rows.stop - rows.start
                        nc.vector.tensor_mul(
                            gt2[rows, h],
                            g2[rows],
                            ap.unsqueeze(1).broadcast_to((nr, L, L)),
                        )
                nc.vector.reduce_sum(vpr, gt2, axis=AX.X)
                for h in range(2):
                    for rows, ap in s_aps(t, s2h, h):
                        nc.vector.tensor_mul(svp[rows, h], ap, vpr[rows, h])
                nc.vector.tensor_reduce(sq2, svp, axis=AX.X, op=OP.add)
                # scale = sq/((1+sq) sqrt(sq+eps)); sqrt via exp(-0.5 ln);
                # ln(J) bias folds away the 1/J baked into xT
                nc.scalar.activation(ln2, sq2, AF.Ln, bias=eps_t[:])
                nc.scalar.activation(rti, ln2, AF.Exp, scale=-0.5, bias=lnj_t[:])
                nc.vector.tensor_scalar_add(sp1, sq2, 1.0)
                nc.vector.reciprocal(r1, sp1)
                nc.vector.tensor_mul(m2a, sq2, r1)
                nc.vector.tensor_mul(m2, m2a, rti)
                nc.vector.tensor_mul(
                    vT_pad[:, :, 0:L],
                    vpr,
                    m2[:].unsqueeze(2).broadcast_to((P, 2, L)),
                )

            def vtr_phase(t):
                """PE transpose of vT_pad; scatter sample n to vblk rows 32n."""
                vtr = vtr_pool.tile([2 * 32, P], bf16, tag="vtr", name="vtr")
                nc.tensor.transpose(
                    vtr, vT_pad[:].rearrange("p h w -> p (h w)"), id_t
                )
                for n in range(NPC):
                    h, n2 = n // 2, n % 2
                    src = vtr[32 * h : 32 * h + L, 64 * n2 : 64 * n2 + 64]
                    dst = vblk[32 * n : 32 * n + L, 64 * n : 64 * n + 64]
                    if n % 2 == 0:
                        nc.vector.tensor_copy(dst, src)
                    else:
                        nc.scalar.copy(dst, src)

            ones_t = singles.tile([P, P], bf16)
            nc.gpsimd.memset(ones_t, 1.0)

            with tc.tile_pool(name="s2a_ps", bufs=1, space="PSUM") as s2a_pool, \
                 tc.tile_pool(name="s2b_ps", bufs=1, space="PSUM") as s2b_pool, \
                 tc.tile_pool(name="vtr_ps", bufs=1, space="PSUM") as vtr_pool:

                # ================= iteration 0 =================
                with tc.tile_pool(name="xtp_ps", bufs=2, space="PSUM") as xtp_pool:
                    s2h0 = a_phase(0)
                    # xT[32n+l, q, p] = x[n, 9p+q, l]/J via 9 PE transposes;
                    # PSUM->SBUF copies alternate vector/scalar
                    for q in range(Q):
                        xtp = xtp_pool.tile([P, P], bf16, tag="xtp", name="xtp")
                        nc.tensor.transpose(
                            xtp, xpad[:, q].rearrange("p n w -> p (n w)"), id_t
                        )
                        if q % 2 == 0:
                            nc.vector.tensor_copy(xT_sb[:, q, :], xtp)
                        else:
                            nc.scalar.copy(xT_sb[:, q, :], xtp)
                    v_phase(0, s2h0)
                    vtr_phase(0)

                # ================= iterations 1..2 =================
                # logits as 5 one-bank PSUM tiles (q pairs) so the exp of early
                # pairs only waits on their own (b) matmuls
                with tc.tile_pool(name="lp_ps", bufs=1, space="PSUM") as lp_pool:
                    lp = []
                    for g, nq in groups():
                        lpt = lp_pool.tile(
                            [P, nq, NPC, J], f32, tag=f"lp{g}", name=f"lp{g}"
                        )
                        lp.append(lpt)

                    for t in range(1, ITERS):
                        # ---- (b) matmuls with prev iter's vblk ----
                        for q in range(Q):
                            g, qq = q // 2, q % 2
                            nc.tensor.matmul(
                                lp[g][:, qq].rearrange("p n j -> p (n j)"),
                                xT_sb[:, q, :],
                                vblk[:],
                                start=(t == 1 and qq == 0),
                                stop=(t == ITERS - 1 and (qq == 1 or q == Q - 1)),
                                skip_group_check=True,
                            )
                        # ---- softmax per q-pair group: exp, d, r, xpad=x*r ----
                        for g, nq in groups():
                            sl = slice(2 * g, 2 * g + nq)
                            nc.scalar.activation(e_t[:, sl], lp[g][:], AF.Exp)
                            nc.vector.tensor_reduce(
                                d_t[:, sl], e_t[:, sl], axis=AX.X, op=OP.add
                            )
                            nc.vector.reciprocal(r_t[:, sl], d_t[:, sl])
                            nc.gpsimd.tensor_mul(
                                xpad[:, sl, :, 0:L],
                                xall[:, :, sl, :].transpose([0, 2, 1, 3]),
                                r_t[:, sl]
                                .unsqueeze(3)
                                .broadcast_to((P, nq, NPC, L)),
                            )

                        s2h = a_phase(t)

                        if t < ITERS - 1:
                            v_phase(t, s2h)
                            vtr_phase(t)
                        else:
                            # ---- final: sq via G (feeds scalar), u = W s ----
                            for h in range(2):
                                nc.vector.tensor_mul(
                                    gt2[:, h],
                                    g2[:],
                                    s2h[h][:, 0:L]
                                    .unsqueeze(1)
                                    .broadcast_to((P, L, L)),
                                )
                            nc.vector.reduce_sum(vpr, gt2, axis=AX.X)
                            for h in range(2):
                                nc.vector.tensor_mul(
                                    svp[:, h], s2h[h][:, 0:L], vpr[:, h]
                                )
                            nc.vector.tensor_reduce(sq2, svp, axis=AX.X, op=OP.add)
                            nc.scalar.activation(ln2, sq2, AF.Ln, bias=eps_t[:])
                            nc.scalar.activation(rti, ln2, AF.Exp, scale=-0.5)
                            for h in range(2):
                                nc.vector.tensor_mul(
                                    pt2[:, h],
                                    w2[:],
                                    s2h[h][:, 0:L]
                                    .unsqueeze(1)
                                    .broadcast_to((P, K, L)),
                                )
                            nc.vector.reduce_sum(u2, pt2, axis=AX.X)
                            nc.vector.tensor_scalar_add(sp1, sq2, 1.0)
                            nc.vector.reciprocal(r1, sp1)
                            nc.vector.tensor_mul(m2a, sq2, r1)
                            nc.vector.tensor_mul(m2, m2a, rti)
                            nc.vector.tensor_mul(
                                oc, u2, m2[:].unsqueeze(2).broadcast_to((P, 2, K))
                            )
                            # oc[(n2 j), h, k] -> out[n, j, k], n = 2h + n2
                            nc.sync.dma_start(
                                out=o_d[:].rearrange(
                                    "(h n2) j k -> (n2 j) h k", h=2
                                ),
                                in_=oc,
                            )

    nc.finalize()
    return nc


def _warm_axon():
    """Run one trivial op on the axon devices so the PJRT client (and the
    NTFF profile sidechannel) is fully initialized before the traced run."""
    if _cache.get("warm"):
        return
    try:
        import jax
        import jax.numpy as jnp

        d = jax.devices()[0]
        jnp.add(jax.device_put(jnp.ones((1,), jnp.float32), d), 1.0).block_until_ready()
    except Exception:
        pass
    _cache["warm"] = True


def kernel(x, weight):
    global LAST_RESULT
    from concourse.bass_utils import run_bass_kernel_spmd

    if "nc" not in _cache:
        _cache["nc"] = _build()
    nc = _cache["nc"]
    _warm_axon()

    import ml_dtypes

    x = np.ascontiguousarray(np.asarray(x, dtype=np.float32))
    weight = np.ascontiguousarray(np.asarray(weight, dtype=np.float32))
    gram = np.einsum("jkl,jkm->jlm", weight, weight)
    bf = ml_dtypes.bfloat16
    w_rep = np.ascontiguousarray(np.concatenate([weight, weight], axis=0).astype(bf))
    g_rep = np.ascontiguousarray(np.concatenate([gram, gram], axis=0).astype(bf))

    in_maps = [
        {"x": x[c * NPC : (c + 1) * NPC], "weight": w_rep, "gram": g_rep}
        for c in range(NCORES)
    ]
    last_exc = None
    for attempt in range(3):
        try:
            res = run_bass_kernel_spmd(nc, in_maps, core_ids=list(range(NCORES)))
            break
        except Exception as e:
            last_exc = e
            import time

            time.sleep(5 * (attempt + 1))
    else:
        raise last_exc
    LAST_RESULT = res
    return np.concatenate([r["out"] for r in res.results], axis=0)


# revision 29
# speedup vs baseline: 1.0612x; 1.0235x over previous
"""CapsuleLinear dynamic-routing kernel for TRN2 (8 NeuronCores, data-parallel over batch).

Math (reference):
    priors[n,j,i,k] = sum_l x[n,i,l] * w[j,k,l]          (never materialized)
    3 routing iterations entirely in the L=8 compressed space:
      probs = softmax_j(logits)                          logits[n,i,j], init 0
      s[n,j,l]  = sum_i probs[n,j,i] * x[n,i,l]          (PE matmul, contraction over i)
      v'[n,j,l] = sum_l' G[j,l,l'] s[n,j,l']             (G = W^T W, host-precomputed)
      sq[n,j]   = s . v'  (= ||W s||^2)
      v[n,j,l]  = scale(sq) * v'                         (squash scale folded in)
      logits   += sum_l x[n,i,l] * v[n,j,l]              (PE matmul, PSUM-resident accum)
    Final iter only: u = W s, out = scale(sq) * u.

Layout: i = 9*p + q  (p = SBUF partition 0..127, q = 0..8).
Per-sample tensors are packed (n2, j) on 128 partitions (n = 2h + n2, h = sample
half).  The (a) matmuls are emitted one-per-sample (4 per q) so s lands
full-partition-width at column 0 of its PSUM bank — the squash chain reads it
straight from PSUM with no diagonal-extract copies.
logits lives in 5 PSUM tiles of one bank each (q pairs); softmax d/r/xs are
processed per q-pair group so the exp of early groups feeds (a) matmuls while
later groups are still in flight.  Softmax normalization is folded into xpad
(x * r per i); x/J in bf16 for iter 0 (J folded back via the Exp bias ln(J)).
Matmul operands are bf16; all PSUM accumulation is f32.
"""

import os

import numpy as np

N, I, L, J, K = 32, 1152, 8, 64, 16
NCORES = 8
NPC = N // NCORES  # samples per core = 4
P = 128
Q = I // P  # 9
ITERS = 3
EPS = 1e-9
LN_J = float(np.log(float(J)))
NG = 5  # exp/logit PSUM groups: q pairs (0,1),(2,3),(4,5),(6,7),(8,)

_cache = {}
LAST_RESULT = None


def _patch_act_tables():
    """Restrict every activation to the one table set containing Exp+Ln
    (+Copy/Identity/Square), so bacc emits a single ACT_TABLE_LOAD."""
    import concourse.hw_specs as hw_specs

    import concourse.bacc as bacc

    if getattr(hw_specs, "_capsule_patched", False):
        return
    orig = hw_specs.get_activation_tables

    def patched(arch):
        t = dict(orig(arch))
        both = "natural_log_exp_and_others"
        if both in t:
            keep = t[both]
            for name in t:
                if name != both:
                    t[name] = t[name] - keep
        return t

    hw_specs.get_activation_tables = patched
    bacc.get_activation_tables = patched  # bacc binds the name via from-import
    hw_specs._capsule_patched = True


def _build():
    import concourse.bacc as bacc
    import concourse.tile as tile
    from concourse import mybir
    from concourse.masks import make_identity

    _patch_act_tables()

    f32 = mybir.dt.float32
    bf16 = mybir.dt.bfloat16
    AF = mybir.ActivationFunctionType
    AX = mybir.AxisListType
    OP = mybir.AluOpType
    dlow = bool(int(os.environ.get("DLOW", "0")))
    ddt = bf16 if dlow else f32

    nc = bacc.Bacc("TRN2", target_bir_lowering=False, debug=False, num_devices=NCORES)

    x_d = nc.dram_tensor("x", (NPC, I, L), f32, kind="ExternalInput")
    # weight/gram replicated to 128 partitions host-side, shipped bf16
    w_d = nc.dram_tensor("weight", (P, K, L), bf16, kind="ExternalInput")
    g_d = nc.dram_tensor("gram", (P, L, L), bf16, kind="ExternalInput")
    o_d = nc.dram_tensor("out", (NPC, J, K), f32, kind="ExternalOutput")

    with tile.TileContext(nc) as tc:
        with tc.tile_pool(name="singles", bufs=1) as singles:
            # ---- input DMAs: 2-way sample split; w/G single-trigger ----
            xall = singles.tile([P, NPC, Q, L], f32)
            xsrc = x_d[:].rearrange("n (p q) l -> p n q l", p=P)
            nc.sync.dma_start(out=xall[:, 0:2], in_=xsrc[:, 0:2])
            nc.scalar.dma_start(out=xall[:, 2:4], in_=xsrc[:, 2:4])
            w2 = singles.tile([P, K, L], bf16)
            nc.sync.dma_start(out=w2[:], in_=w_d[:])
            g2 = singles.tile([P, L, L], bf16)
            nc.scalar.dma_start(out=g2[:], in_=g_d[:])

            # ---- constants (xpad memset first: it gates the xs converts) ----
            xpad = singles.tile([P, Q, NPC, 32], bf16)
            nc.gpsimd.memset(xpad, 0.0)
            id_t = singles.tile([P, P], bf16)
            make_identity(nc, id_t)
            vT_pad = singles.tile([P, 2, 32], bf16)
            nc.gpsimd.memset(vT_pad, 0.0)
            # block-diag v: sample n occupies partitions 32n..32n+8
            vblk = singles.tile([P, NPC * J], bf16)
            nc.gpsimd.memset(vblk, 0.0)
            eps_t = singles.tile([P, 1], f32)
            nc.gpsimd.memset(eps_t, EPS)
            lnj_t = singles.tile([P, 1], f32)
            nc.gpsimd.memset(lnj_t, LN_J)

            # warm the ACT ln/exp table set while DMAs run
            warm = singles.tile([1, 2], f32)
            nc.gpsimd.memset(warm, 1.0)
            nc.scalar.activation(warm[:, 0:1], warm[:, 0:1], AF.Ln)
            nc.scalar.activation(warm[:, 1:2], warm[:, 1:2], AF.Exp)

            # ---- per-sample bf16 conversion (iter-0 probs folded: x/J) ----
            for n in range(NPC):
                nc.vector.tensor_scalar_mul(
                    xpad[:, :, n : n + 1, 0:L],
                    xall[:, n : n + 1].transpose([0, 2, 1, 3]),
                    1.0 / J,
                )

            # ---- iteration temporaries ----
            xT_sb = singles.tile([P, Q, P], bf16)
            e_t = singles.tile([P, Q, NPC, J], bf16)
            d_t = singles.tile([P, Q, NPC], ddt)
            r_t = singles.tile([P, Q, NPC], ddt)
            gt2 = singles.tile([P, 2, L, L], f32)
            vpr = singles.tile([P, 2, L], f32)
            svp = singles.tile([P, 2, L], f32)
            sq2 = singles.tile([P, 2], f32)
            ln2 = singles.tile([P, 2], f32)
            rti = singles.tile([P, 2], f32)
            sp1 = singles.tile([P, 2], f32)
            r1 = singles.tile([P, 2], f32)
            m2a = singles.tile([P, 2], f32)
            m2 = singles.tile([P, 2], f32)
            pt2 = singles.tile([P, 2, K, L], f32)
            u2 = singles.tile([P, 2, K], f32)
            oc = singles.tile([P, 2, K], f32)

            def groups():
                for g in range(NG):
                    yield g, (2 if g < 4 else 1)

            def a_phase(t):
                """(a) matmuls into s2a/s2b PSUM banks.
                t=0: probs uniform -> 2 wide ones-matmuls per q (s packed
                (n2, l) in 16 cols, diag extracted by the v-chain APs).
                t>=1: one matmul per sample (4 per q) so s lands at cols 0:L
                across the full partition width (no diag extract at all)."""
                s2h = [
                    s2a_pool.tile([P, 2 * L], f32, tag="s2a", name="s2a"),
                    s2b_pool.tile([P, 2 * L], f32, tag="s2b", name="s2b"),
                ]
                for h in range(2):
                    for q in range(Q):
                        if t == 0:
                            nc.tensor.matmul(
                                s2h[h],
                                ones_t[:],
                                xpad[:, q, 2 * h : 2 * h + 2, 0:L],
                                start=(q == 0),
                                stop=(q == Q - 1),
                                skip_group_check=True,
                            )
                        else:
                            for n2 in range(2):
                                n = 2 * h + n2
                                nc.tensor.matmul(
                                    s2h[h][64 * n2 : 64 * n2 + 64, 0:L],
                                    e_t[:, q, n, :],
                                    xpad[:, q, n, 0:L],
                                    start=(q == 0),
                                    stop=(q == Q - 1),
                                    skip_group_check=True,
                                )
                return s2h

            def s_aps(t, s2h, h):
                """Where s for half h lives in PSUM: (row-slice, AP) pieces."""
                if t == 0:
                    return [
                        (slice(0, 64), s2h[h][0:64, 0:L]),
                        (slice(64, 128), s2h[h][64:128, L : 2 * L]),
                    ]
                return [(slice(0, P), s2h[h][:, 0:L])]

            def v_phase(t, s2h):
                """v' = G s, sq = s.v', scale chain into vT_pad (scale*J).
                Vector-centric; scalar only does Ln/Exp."""
                for h in range(2):
                    for rows, ap in s_aps(t, s2h, h):
                        nr = rows.stop - rows.start
                        nc.vector.tensor_mul(
                            gt2[rows, h],
                            g2[rows],
                            ap.unsqueeze(1).broadcast_to((nr, L, L)),
                        )
                nc.vector.reduce_sum(vpr, gt2, axis=AX.X)
                for h in range(2):
                    for rows, ap in s_aps(t, s2h, h):
                        # svp = s*v', sq = sum(svp) fused in one DVE op
                        nc.vector.scalar_tensor_tensor(
                            svp[rows, h],
                            ap,
                            0.0,
                            vpr[rows, h],
                            OP.bypass,
                            OP.mult,
                            accum_out=sq2[rows, h : h + 1],
                        )
                # scale = sq/((1+sq) sqrt(sq+eps)); sqrt via exp(-0.5 ln);
                # ln(J) bias folds away the 1/J baked into xT
                nc.scalar.activation(ln2, sq2, AF.Ln, bias=eps_t[:])
                nc.scalar.activation(rti, ln2, AF.Exp, scale=-0.5, bias=lnj_t[:])
                nc.vector.tensor_scalar_add(sp1, sq2, 1.0)
                nc.vector.reciprocal(r1, sp1)
                nc.vector.tensor_mul(m2a, sq2, r1)
                nc.vector.tensor_mul(m2, m2a, rti)
                nc.vector.tensor_mul(
                    vT_pad[:, :, 0:L],
                    vpr,
                    m2[:].unsqueeze(2).broadcast_to((P, 2, L)),
                )

            def vtr_phase(t):
                """PE transpose of vT_pad; scatter sample n to vblk rows 32n."""
                vtr = vtr_pool.tile([2 * 32, P], bf16, tag="vtr", name="vtr")
                nc.tensor.transpose(
                    vtr, vT_pad[:].rearrange("p h w -> p (h w)"), id_t
                )
                for n in range(NPC):
                    h, n2 = n // 2, n % 2
                    src = vtr[32 * h : 32 * h + L, 64 * n2 : 64 * n2 + 64]
                    dst = vblk[32 * n : 32 * n + L, 64 * n : 64 * n + 64]
                    if n % 2 == 0:
                        nc.vector.tensor_copy(dst, src)
                    else:
                        nc.scalar.copy(dst, src)

            ones_t = singles.tile([P, P], bf16)
            nc.gpsimd.memset(ones_t, 1.0)

            with tc.tile_pool(name="s2a_ps", bufs=1, space="PSUM") as s2a_pool, \
                 tc.tile_pool(name="s2b_ps", bufs=1, space="PSUM") as s2b_pool, \
                 tc.tile_pool(name="vtr_ps", bufs=1, space="PSUM") as vtr_pool:

                # ================= iteration 0 =================
                with tc.tile_pool(name="xtp_ps", bufs=3, space="PSUM") as xtp_pool:
                    s2h0 = a_phase(0)
                    # xT[32n+l, q, p] = x[n, 9p+q, l]/J via 9 PE transposes;
                    # PSUM->SBUF copies alternate vector/scalar
                    for q in range(Q):
                        xtp = xtp_pool.tile([P, P], bf16, tag="xtp", name="xtp")
                        nc.tensor.transpose(
                            xtp, xpad[:, q].rearrange("p n w -> p (n w)"), id_t
                        )
                        if q % 3 == 0:
                            nc.vector.tensor_copy(xT_sb[:, q, :], xtp)
                        else:
                            nc.scalar.copy(xT_sb[:, q, :], xtp)
                    v_phase(0, s2h0)
                    vtr_phase(0)

                # ================= iterations 1..2 =================
                # logits as 5 one-bank PSUM tiles (q pairs) so the exp of early
                # pairs only waits on their own (b) matmuls
                with tc.tile_pool(name="lp_ps", bufs=1, space="PSUM") as lp_pool:
                    lp = []
                    for g, nq in groups():
                        lpt = lp_pool.tile(
                            [P, nq, NPC, J], f32, tag=f"lp{g}", name=f"lp{g}"
                        )
                        lp.append(lpt)

                    for t in range(1, ITERS):
                        # ---- (b) matmuls with prev iter's vblk ----
                        for q in range(Q):
                            g, qq = q // 2, q % 2
                            nc.tensor.matmul(
                                lp[g][:, qq].rearrange("p n j -> p (n j)"),
                                xT_sb[:, q, :],
                                vblk[:],
                                start=(t == 1 and qq == 0),
                                stop=(t == ITERS - 1 and (qq == 1 or q == Q - 1)),
                                skip_group_check=True,
                            )
                        # ---- softmax per q-pair group: exp, d, r, xpad=x*r ----
                        for g, nq in groups():
                            sl = slice(2 * g, 2 * g + nq)
                            nc.scalar.activation(e_t[:, sl], lp[g][:], AF.Exp)
                            nc.vector.tensor_reduce(
                                d_t[:, sl], e_t[:, sl], axis=AX.X, op=OP.add
                            )
                            nc.vector.reciprocal(r_t[:, sl], d_t[:, sl])
                            nc.gpsimd.tensor_mul(
                                xpad[:, sl, :, 0:L],
                                xall[:, :, sl, :].transpose([0, 2, 1, 3]),
                                r_t[:, sl]
                                .unsqueeze(3)
                                .broadcast_to((P, nq, NPC, L)),
                            )

                        s2h = a_phase(t)

                        if t < ITERS - 1:
                            v_phase(t, s2h)
                            vtr_phase(t)
                        else:
                            # ---- final: sq via G (feeds scalar), u = W s ----
                            for h in range(2):
                                nc.vector.tensor_mul(
                                    gt2[:, h],
                                    g2[:],
                                    s2h[h][:, 0:L]
                                    .unsqueeze(1)
                                    .broadcast_to((P, L, L)),
                                )
                            nc.vector.reduce_sum(vpr, gt2, axis=AX.X)
                            for h in range(2):
                                nc.vector.scalar_tensor_tensor(
                                    svp[:, h],
                                    s2h[h][:, 0:L],
                                    0.0,
                                    vpr[:, h],
                                    OP.bypass,
                                    OP.mult,
                                    accum_out=sq2[:, h : h + 1],
                                )
                            nc.scalar.activation(ln2, sq2, AF.Ln, bias=eps_t[:])
                            nc.scalar.activation(rti, ln2, AF.Exp, scale=-0.5)
                            for h in range(2):
                                nc.vector.tensor_mul(
                                    pt2[:, h],
                                    w2[:],
                                    s2h[h][:, 0:L]
                                    .unsqueeze(1)
                                    .broadcast_to((P, K, L)),
                                )
                            nc.vector.reduce_sum(u2, pt2, axis=AX.X)
                            nc.vector.tensor_scalar_add(sp1, sq2, 1.0)
                            nc.vector.reciprocal(r1, sp1)
                            nc.vector.tensor_mul(m2a, sq2, r1)
                            nc.vector.tensor_mul(m2, m2a, rti)
                            nc.vector.tensor_mul(
                                oc, u2, m2[:].unsqueeze(2).broadcast_to((P, 2, K))
                            )
                            # oc[(n2 j), h, k] -> out[n, j, k], n = 2h + n2
                            nc.sync.dma_start(
                                out=o_d[:].rearrange(
                                    "(h n2) j k -> (n2 j) h k", h=2
                                ),
                                in_=oc,
                            )

    nc.finalize()
    return nc


def _warm_axon():
    """Run one trivial op on the axon devices so the PJRT client (and the
    NTFF profile sidechannel) is fully initialized before the traced run."""
    if _cache.get("warm"):
        return
    try:
        import jax
        import jax.numpy as jnp

        d = jax.devices()[0]
        jnp.add(jax.device_put(jnp.ones((1,), jnp.float32), d), 1.0).block_until_ready()
    except Exception:
        pass
    _cache["warm"] = True


def kernel(x, weight):
    global LAST_RESULT
    from concourse.bass_utils import run_bass_kernel_spmd

    if "nc" not in _cache:
        _cache["nc"] = _build()
    nc = _cache["nc"]
    _warm_axon()

    import ml_dtypes

    x = np.ascontiguousarray(np.asarray(x, dtype=np.float32))
    weight = np.ascontiguousarray(np.asarray(weight, dtype=np.float32))
    gram = np.einsum("jkl,jkm->jlm", weight, weight)
    bf = ml_dtypes.bfloat16
    w_rep = np.ascontiguousarray(np.concatenate([weight, weight], axis=0).astype(bf))
    g_rep = np.ascontiguousarray(np.concatenate([gram, gram], axis=0).astype(bf))

    in_maps = [
        {"x": x[c * NPC : (c + 1) * NPC], "weight": w_rep, "gram": g_rep}
        for c in range(NCORES)
    ]
    last_exc = None
    for attempt in range(3):
        try:
            res = run_bass_kernel_spmd(nc, in_maps, core_ids=list(range(NCORES)))
            break
        except Exception as e:
            last_exc = e
            import time

            time.sleep(5 * (attempt + 1))
    else:
        raise last_exc
    LAST_RESULT = res
    return np.concatenate([r["out"] for r in res.results], axis=0)


# revision 33
# speedup vs baseline: 1.0658x; 1.0044x over previous
"""CapsuleLinear dynamic-routing kernel for TRN2 (8 NeuronCores, data-parallel over batch).

Math (reference):
    priors[n,j,i,k] = sum_l x[n,i,l] * w[j,k,l]          (never materialized)
    3 routing iterations entirely in the L=8 compressed space:
      probs = softmax_j(logits)                          logits[n,i,j], init 0
      s[n,j,l]  = sum_i probs[n,j,i] * x[n,i,l]          (PE matmul, contraction over i)
      v'[n,j,l] = sum_l' G[j,l,l'] s[n,j,l']             (G = W^T W, host-precomputed)
      sq[n,j]   = s . v'  (= ||W s||^2)
      v[n,j,l]  = scale(sq) * v'                         (squash scale folded in)
      logits   += sum_l x[n,i,l] * v[n,j,l]              (PE matmul, PSUM-resident accum)
    Final iter only: u = W s, out = scale(sq) * u.

Layout: i = 9*p + q  (p = SBUF partition 0..127, q = 0..8).
Per-sample tensors are packed (n2, j) on 128 partitions (n = 2h + n2, h = sample
half).  The (a) matmuls are emitted one-per-sample (4 per q) so s lands
full-partition-width at column 0 of its PSUM bank — the squash chain reads it
straight from PSUM with no diagonal-extract copies.
logits lives in 5 PSUM tiles of one bank each (q pairs); softmax d/r/xs are
processed per q-pair group so the exp of early groups feeds (a) matmuls while
later groups are still in flight.  Softmax normalization is folded into xpad
(x * r per i); x/J in bf16 for iter 0 (J folded back via the Exp bias ln(J)).
Matmul operands are bf16; all PSUM accumulation is f32.
"""

import os

import numpy as np

N, I, L, J, K = 32, 1152, 8, 64, 16
NCORES = 8
NPC = N // NCORES  # samples per core = 4
P = 128
Q = I // P  # 9
ITERS = 3
EPS = 1e-9
LN_J = float(np.log(float(J)))
NG = 5  # exp/logit PSUM groups: q pairs (0,1),(2,3),(4,5),(6,7),(8,)

_cache = {}
LAST_RESULT = None


def _patch_act_tables():
    """Restrict every activation to the one table set containing Exp+Ln
    (+Copy/Identity/Square), so bacc emits a single ACT_TABLE_LOAD."""
    import concourse.hw_specs as hw_specs

    import concourse.bacc as bacc

    if getattr(hw_specs, "_capsule_patched", False):
        return
    orig = hw_specs.get_activation_tables

    def patched(arch):
        t = dict(orig(arch))
        both = "natural_log_exp_and_others"
        if both in t:
            keep = t[both]
            for name in t:
                if name != both:
                    t[name] = t[name] - keep
        return t

    hw_specs.get_activation_tables = patched
    bacc.get_activation_tables = patched  # bacc binds the name via from-import
    hw_specs._capsule_patched = True


def _build():
    import concourse.bacc as bacc
    import concourse.tile as tile
    from concourse import mybir
    from concourse.masks import make_identity

    _patch_act_tables()

    f32 = mybir.dt.float32
    bf16 = mybir.dt.bfloat16
    AF = mybir.ActivationFunctionType
    AX = mybir.AxisListType
    OP = mybir.AluOpType
    dlow = bool(int(os.environ.get("DLOW", "0")))
    ddt = bf16 if dlow else f32

    nc = bacc.Bacc("TRN2", target_bir_lowering=False, debug=False, num_devices=NCORES)

    x_d = nc.dram_tensor("x", (NPC, I, L), f32, kind="ExternalInput")
    # weight/gram replicated to 128 partitions host-side, shipped bf16
    w_d = nc.dram_tensor("weight", (P, K, L), bf16, kind="ExternalInput")
    g_d = nc.dram_tensor("gram", (P, L, L), bf16, kind="ExternalInput")
    o_d = nc.dram_tensor("out", (NPC, J, K), f32, kind="ExternalOutput")

    with tile.TileContext(nc) as tc:
        with tc.tile_pool(name="singles", bufs=1) as singles:
            # ---- input DMAs: 2-way sample split; w/G single-trigger ----
            xall = singles.tile([P, NPC, Q, L], f32)
            xsrc = x_d[:].rearrange("n (p q) l -> p n q l", p=P)
            nc.sync.dma_start(out=xall[:, 0:2], in_=xsrc[:, 0:2])
            nc.scalar.dma_start(out=xall[:, 2:4], in_=xsrc[:, 2:4])
            w2 = singles.tile([P, K, L], bf16)
            nc.sync.dma_start(out=w2[:], in_=w_d[:])
            g2 = singles.tile([P, L, L], bf16)
            nc.scalar.dma_start(out=g2[:], in_=g_d[:])

            # ---- constants (xpad memset first: it gates the xs converts) ----
            xpad = singles.tile([P, Q, NPC, 32], bf16)
            nc.gpsimd.memset(xpad, 0.0)
            id_t = singles.tile([P, P], bf16)
            make_identity(nc, id_t)
            vT_pad = singles.tile([P, 2, 32], bf16)
            nc.gpsimd.memset(vT_pad, 0.0)
            # block-diag v: sample n occupies partitions 32n..32n+8
            vblk = singles.tile([P, NPC * J], bf16)
            nc.gpsimd.memset(vblk, 0.0)
            eps_t = singles.tile([P, 1], f32)
            nc.gpsimd.memset(eps_t, EPS)
            lnj_t = singles.tile([P, 1], f32)
            nc.gpsimd.memset(lnj_t, LN_J)

            # warm the ACT ln/exp table set while DMAs run
            warm = singles.tile([1, 2], f32)
            nc.gpsimd.memset(warm, 1.0)
            nc.scalar.activation(warm[:, 0:1], warm[:, 0:1], AF.Ln)
            nc.scalar.activation(warm[:, 1:2], warm[:, 1:2], AF.Exp)

            # ---- per-sample bf16 conversion (iter-0 probs folded: x/J) ----
            for n in range(NPC):
                nc.vector.tensor_scalar_mul(
                    xpad[:, :, n : n + 1, 0:L],
                    xall[:, n : n + 1].transpose([0, 2, 1, 3]),
                    1.0 / J,
                )

            # ---- iteration temporaries ----
            xT_sb = singles.tile([P, Q, P], bf16)
            e_t = singles.tile([P, Q, NPC, J], bf16)
            d_t = singles.tile([P, Q, NPC], ddt)
            r_t = singles.tile([P, Q, NPC], ddt)
            gt2 = singles.tile([P, 2, L, L], f32)
            vpr = singles.tile([P, 2, L], f32)
            svp = singles.tile([P, 2, L], f32)
            sq2 = singles.tile([P, 2], f32)
            ln2 = singles.tile([P, 2], f32)
            rti = singles.tile([P, 2], f32)
            sp1 = singles.tile([P, 2], f32)
            r1 = singles.tile([P, 2], f32)
            m2a = singles.tile([P, 2], f32)
            m2 = singles.tile([P, 2], f32)
            pt2 = singles.tile([P, 2, K, L], f32)
            u2 = singles.tile([P, 2, K], f32)
            oc = singles.tile([P, 2, K], f32)

            def groups():
                for g in range(NG):
                    yield g, (2 if g < 4 else 1)

            def a_phase(t):
                """(a) matmuls into s2a/s2b PSUM banks.
                t=0: probs uniform -> 2 wide ones-matmuls per q (s packed
                (n2, l) in 16 cols, diag extracted by the v-chain APs).
                t>=1: one matmul per sample (4 per q) so s lands at cols 0:L
                across the full partition width (no diag extract at all)."""
                s2h = [
                    s2a_pool.tile([P, 2 * L], f32, tag="s2a", name="s2a"),
                    s2b_pool.tile([P, 2 * L], f32, tag="s2b", name="s2b"),
                ]
                for h in range(2):
                    for q in range(Q):
                        if t == 0:
                            nc.tensor.matmul(
                                s2h[h],
                                ones_t[:],
                                xpad[:, q, 2 * h : 2 * h + 2, 0:L],
                                start=(q == 0),
                                stop=(q == Q - 1),
                                skip_group_check=True,
                            )
                        else:
                            for n2 in range(2):
                                n = 2 * h + n2
                                nc.tensor.matmul(
                                    s2h[h][64 * n2 : 64 * n2 + 64, 0:L],
                                    e_t[:, q, n, :],
                                    xpad[:, q, n, 0:L],
                                    start=(q == 0),
                                    stop=(q == Q - 1),
                                    skip_group_check=True,
                                )
                return s2h

            def s_aps(t, s2h, h):
                """Where s for half h lives in PSUM: (row-slice, AP) pieces."""
                if t == 0:
                    return [
                        (slice(0, 64), s2h[h][0:64, 0:L]),
                        (slice(64, 128), s2h[h][64:128, L : 2 * L]),
                    ]
                return [(slice(0, P), s2h[h][:, 0:L])]

            def v_phase(t, s2h):
                """v' = G s, sq = s.v', scale chain into vT_pad (scale*J).
                Vector-centric; scalar only does Ln/Exp."""
                for h in range(2):
                    for rows, ap in s_aps(t, s2h, h):
                        nr = rows.stop - rows.start
                        nc.vector.tensor_mul(
                            gt2[rows, h],
                            g2[rows],
                            ap.unsqueeze(1).broadcast_to((nr, L, L)),
                        )
                nc.vector.reduce_sum(vpr, gt2, axis=AX.X)
                for h in range(2):
                    for rows, ap in s_aps(t, s2h, h):
                        # svp = s*v', sq = sum(svp) fused in one DVE op
                        nc.vector.scalar_tensor_tensor(
                            svp[rows, h],
                            ap,
                            0.0,
                            vpr[rows, h],
                            OP.bypass,
                            OP.mult,
                            accum_out=sq2[rows, h : h + 1],
                        )
                # scale = sq/((1+sq) sqrt(sq+eps)); sqrt via exp(-0.5 ln);
                # ln(J) bias folds away the 1/J baked into xT
                nc.scalar.activation(ln2, sq2, AF.Ln, bias=eps_t[:])
                nc.scalar.activation(rti, ln2, AF.Exp, scale=-0.5, bias=lnj_t[:])
                nc.vector.tensor_scalar_add(sp1, sq2, 1.0)
                nc.vector.reciprocal(r1, sp1)
                nc.vector.tensor_mul(m2a, sq2, r1)
                nc.vector.tensor_mul(m2, m2a, rti)
                nc.vector.tensor_mul(
                    vT_pad[:, :, 0:L],
                    vpr,
                    m2[:].unsqueeze(2).broadcast_to((P, 2, L)),
                )

            def vtr_phase(t):
                """PE transpose of vT_pad; scatter sample n to vblk rows 32n."""
                vtr = vtr_pool.tile([2 * 32, P], bf16, tag="vtr", name="vtr")
                nc.tensor.transpose(
                    vtr, vT_pad[:].rearrange("p h w -> p (h w)"), id_t
                )
                for n in range(NPC):
                    h, n2 = n // 2, n % 2
                    src = vtr[32 * h : 32 * h + L, 64 * n2 : 64 * n2 + 64]
                    dst = vblk[32 * n : 32 * n + L, 64 * n : 64 * n + 64]
                    if n % 2 == 0:
                        nc.vector.tensor_copy(dst, src)
                    else:
                        nc.scalar.copy(dst, src)

            ones_t = singles.tile([P, P], bf16)
            nc.gpsimd.memset(ones_t, 1.0)

            with tc.tile_pool(name="s2a_ps", bufs=1, space="PSUM") as s2a_pool, \
                 tc.tile_pool(name="s2b_ps", bufs=1, space="PSUM") as s2b_pool, \
                 tc.tile_pool(name="vtr_ps", bufs=1, space="PSUM") as vtr_pool:

                # ================= iteration 0 =================
                with tc.tile_pool(name="xtp_ps", bufs=3, space="PSUM") as xtp_pool:
                    def xT_transpose(q):
                        """xT[32n+l, q, p] = x[n, 9p+q, l]/J via a PE transpose;
                        PSUM->SBUF copy alternates vector/scalar."""
                        xtp = xtp_pool.tile([P, P], bf16, tag="xtp", name="xtp")
                        nc.tensor.transpose(
                            xtp, xpad[:, q].rearrange("p n w -> p (n w)"), id_t
                        )
                        if q % 3 == 0:
                            nc.vector.tensor_copy(xT_sb[:, q, :], xtp)
                        else:
                            nc.scalar.copy(xT_sb[:, q, :], xtp)

                    s2h0 = a_phase(0)
                    # early transposes feed iter1's first (b) matmuls; the
                    # rest are deferred past vtr so the scatter copies are
                    # not stuck behind them in the scalar/vector queues
                    for q in range(5):
                        xT_transpose(q)
                    v_phase(0, s2h0)
                    vtr_phase(0)
                    for q in range(5, Q):
                        xT_transpose(q)

                # ================= iterations 1..2 =================
                # logits as 5 one-bank PSUM tiles (q pairs) so the exp of early
                # pairs only waits on their own (b) matmuls
                with tc.tile_pool(name="lp_ps", bufs=1, space="PSUM") as lp_pool:
                    lp = []
                    for g, nq in groups():
                        lpt = lp_pool.tile(
                            [P, nq, NPC, J], f32, tag=f"lp{g}", name=f"lp{g}"
                        )
                        lp.append(lpt)

                    for t in range(1, ITERS):
                        # ---- (b) matmuls with prev iter's vblk ----
                        for q in range(Q):
                            g, qq = q // 2, q % 2
                            nc.tensor.matmul(
                                lp[g][:, qq].rearrange("p n j -> p (n j)"),
                                xT_sb[:, q, :],
                                vblk[:],
                                start=(t == 1 and qq == 0),
                                stop=(t == ITERS - 1 and (qq == 1 or q == Q - 1)),
                                skip_group_check=True,
                            )
                        # ---- softmax per q-pair group: exp, d, r, xpad=x*r ----
                        for g, nq in groups():
                            sl = slice(2 * g, 2 * g + nq)
                            nc.scalar.activation(e_t[:, sl], lp[g][:], AF.Exp)
                            with nc.allow_low_precision("softmax denom in bf16"):
                                nc.vector.tensor_reduce(
                                    d_t[:, sl], e_t[:, sl], axis=AX.X, op=OP.add
                                )
                                nc.vector.reciprocal(r_t[:, sl], d_t[:, sl])
                            nc.gpsimd.tensor_mul(
                                xpad[:, sl, :, 0:L],
                                xall[:, :, sl, :].transpose([0, 2, 1, 3]),
                                r_t[:, sl]
                                .unsqueeze(3)
                                .broadcast_to((P, nq, NPC, L)),
                            )

                        s2h = a_phase(t)

                        if t < ITERS - 1:
                            v_phase(t, s2h)
                            vtr_phase(t)
                        else:
                            # ---- final: sq via G (feeds scalar), u = W s ----
                            for h in range(2):
                                nc.vector.tensor_mul(
                                    gt2[:, h],
                                    g2[:],
                                    s2h[h][:, 0:L]
                                    .unsqueeze(1)
                                    .broadcast_to((P, L, L)),
                                )
                            nc.vector.reduce_sum(vpr, gt2, axis=AX.X)
                            for h in range(2):
                                nc.vector.scalar_tensor_tensor(
                                    svp[:, h],
                                    s2h[h][:, 0:L],
                                    0.0,
                                    vpr[:, h],
                                    OP.bypass,
                                    OP.mult,
                                    accum_out=sq2[:, h : h + 1],
                                )
                            nc.scalar.activation(ln2, sq2, AF.Ln, bias=eps_t[:])
                            nc.scalar.activation(rti, ln2, AF.Exp, scale=-0.5)
                            # recip chain ahead of the u computation in the
                            # vector queue so m2 is ready when Exp lands
                            nc.vector.tensor_scalar_add(sp1, sq2, 1.0)
                            nc.vector.reciprocal(r1, sp1)
                            nc.vector.tensor_mul(m2a, sq2, r1)
                            for h in range(2):
                                nc.vector.tensor_mul(
                                    pt2[:, h],
                                    w2[:],
                                    s2h[h][:, 0:L]
                                    .unsqueeze(1)
                                    .broadcast_to((P, K, L)),
                                )
                            nc.vector.reduce_sum(u2, pt2, axis=AX.X)
                            nc.vector.tensor_mul(m2, m2a, rti)
                            nc.vector.tensor_mul(
                                oc, u2, m2[:].unsqueeze(2).broadcast_to((P, 2, K))
                            )
                            # oc[(n2 j), h, k] -> out[n, j, k], n = 2h + n2
                            odst = o_d[:].rearrange("(h n2) j k -> (n2 j) h k", h=2)
                            nc.sync.dma_start(out=odst[:, 0:1], in_=oc[:, 0:1])
                            nc.scalar.dma_start(out=odst[:, 1:2], in_=oc[:, 1:2])

    nc.finalize()
    return nc


def _warm_axon():
    """Run one trivial op on the axon devices so the PJRT client (and the
    NTFF profile sidechannel) is fully initialized before the traced run."""
    if _cache.get("warm"):
        return
    try:
        import jax
        import jax.numpy as jnp

        d = jax.devices()[0]
        jnp.add(jax.device_put(jnp.ones((1,), jnp.float32), d), 1.0).block_until_ready()
    except Exception:
        pass
    _cache["warm"] = True


def kernel(x, weight):
    global LAST_RESULT
    from concourse.bass_utils import run_bass_kernel_spmd

    if "nc" not in _cache:
        _cache["nc"] = _build()
    nc = _cache["nc"]
    _warm_axon()

    import ml_dtypes

    x = np.ascontiguousarray(np.asarray(x, dtype=np.float32))
    weight = np.ascontiguousarray(np.asarray(weight, dtype=np.float32))
    gram = np.einsum("jkl,jkm->jlm", weight, weight)
    bf = ml_dtypes.bfloat16
    w_rep = np.ascontiguousarray(np.concatenate([weight, weight], axis=0).astype(bf))
    g_rep = np.ascontiguousarray(np.concatenate([gram, gram], axis=0).astype(bf))

    in_maps = [
        {"x": x[c * NPC : (c + 1) * NPC], "weight": w_rep, "gram": g_rep}
        for c in range(NCORES)
    ]
    last_exc = None
    for attempt in range(3):
        try:
            res = run_bass_kernel_spmd(nc, in_maps, core_ids=list(range(NCORES)))
            break
        except Exception as e:
            last_exc = e
            import time

            time.sleep(5 * (attempt + 1))
    else:
        raise last_exc
    LAST_RESULT = res
    return np.concatenate([r["out"] for r in res.results], axis=0)
